# revision 1
# baseline (speedup 1.0000x reference)
"""Multi-head attention (B=4, N=1370, C=1024, H=16) on 8 TRN2 NeuronCores.

Sharding: core = 2*b + g  (b = batch 0..3, g = head-group 0..1 of 8 heads).
Each core: QKV projection for its 8 heads (fp16 matmuls, fp32 accum),
RoPE via a signed-permutation matmul + DVE elementwise, attention with
scores kept transposed [ktok, qtok] so softmax-exp (ACT, PSUM->SBUF) and
attn@v need no transposes, denominators via 64 ones-columns packed into
the v stationary operand, pairwise AllGather of head outputs, then the
projection split by output channels (each core owns 512 of 1024 cols).

Host side only shards / transposes / casts inputs and concatenates the
per-core outputs.
"""

import numpy as np

B, N, C, H, DH = 4, 1370, 1024, 16, 64
P = 128
NH = 685  # qtok half

TOKBLOCKS = [(i * P, P) for i in range(10)] + [(1280, 90)]
CH1370 = [(0, 512), (512, 512), (1024, 346)]
CH1369 = [(0, 512), (512, 512), (1024, 345)]
CH685 = [(0, 512), (512, 173)]

_cached_nc = None


def _build_nc():
    import concourse.bass as bass
    import concourse.mybir as mybir
    import concourse.tile as tile
    from concourse import bacc

    mdt = mybir.dt
    F16, F32, BF16 = mdt.float16, mdt.float32, mdt.bfloat16
    AF = mybir.ActivationFunctionType

    import os
    phases = int(os.environ.get('KERNEL_PHASES', '3'))
    nc = bacc.Bacc(num_devices=8)

    xt_d = nc.declare_dram_parameter("xt", [C, N], F16, isOutput=False)
    wq_d = nc.declare_dram_parameter("wq", [C, 512], F16, isOutput=False)
    wk_d = nc.declare_dram_parameter("wk", [C, 512], F16, isOutput=False)
    wv_d = nc.declare_dram_parameter("wv", [C, 512], F16, isOutput=False)
    bq_d = nc.declare_dram_parameter("bq", [4, P, 1], F32, isOutput=False)
    bk_d = nc.declare_dram_parameter("bk", [4, P, 1], F32, isOutput=False)
    bv_d = nc.declare_dram_parameter("bv", [1, 512], F16, isOutput=False)
    sin_d = nc.declare_dram_parameter("sint", [P, N - 1], F16, isOutput=False)
    cos_d = nc.declare_dram_parameter("cost", [P, N - 1], F16, isOutput=False)
    rm_d = nc.declare_dram_parameter("rmat", [P, P], F16, isOutput=False)
    wp_d = nc.declare_dram_parameter("wp", [C, 512], F16, isOutput=False)
    bp_d = nc.declare_dram_parameter("bp", [1, 512], F16, isOutput=False)
    out_d = nc.declare_dram_parameter("out", [N, 512], F32, isOutput=True)

    xt_r = xt_d.rearrange("(j p) n -> j p n", p=P)
    wq_r = wq_d.rearrange("(j p) n -> j p n", p=P)
    wk_r = wk_d.rearrange("(j p) n -> j p n", p=P)
    wv_r = wv_d.rearrange("(j p) n -> j p n", p=P)
    wp_r = wp_d.rearrange("(j p) n -> j p n", p=P)

    _dma_engines = [nc.sync, nc.gpsimd, nc.scalar, nc.sync, nc.gpsimd]
    _dma_i = [0]

    def dma(out_ap, in_ap):
        e = _dma_engines[_dma_i[0] % len(_dma_engines)]
        _dma_i[0] += 1
        e.dma_start(out_ap, in_ap)

    with tile.TileContext(nc) as tc:
        with (
            tc.tile_pool(name="const", bufs=1) as cp,
            tc.tile_pool(name="qkv", bufs=1) as qp,
            tc.tile_pool(name="vaug", bufs=1) as vp,
            tc.tile_pool(name="hot", bufs=1) as hp_pool,
            tc.tile_pool(name="dram", bufs=1, space="DRAM") as dp,
        ):
            # ---- constants / small inputs ----
            sin_sb = cp.tile([P, N - 1], F16, tag="sin")
            cos_sb = cp.tile([P, N - 1], F16, tag="cos")
            rm_sb = cp.tile([P, P], F16, tag="rm")
            bv_sb = cp.tile([1, 512], F16, tag="bv")
            bp_sb = cp.tile([1, 512], F16, tag="bp")
            ones_sb = cp.tile([1, P], F16, tag="ones")
            dma(sin_sb[:, :], sin_d[:, :])
            dma(cos_sb[:, :], cos_d[:, :])
            dma(rm_sb[:, :], rm_d[:, :])
            dma(bv_sb[:, :], bv_d[:, :])
            dma(bp_sb[:, :], bp_d[:, :])
            nc.gpsimd.memset(ones_sb[:, :], 1.0)
            bq_sb = []
            bk_sb = []
            for hp in range(4):
                tq = cp.tile([P, 1], F32, tag=f"bq{hp}")
                tk = cp.tile([P, 1], F32, tag=f"bk{hp}")
                dma(tq[:, :], bq_d[hp, :, :])
                dma(tk[:, :], bk_d[hp, :, :])
                bq_sb.append(tq)
                bk_sb.append(tk)

            # persistent activations
            qb_sb = [qp.tile([P, N], F16, tag=f"qb{i}", name=f"qb{i}") for i in range(4)]
            kb_sb = [qp.tile([P, N], F16, tag=f"kb{i}", name=f"kb{i}") for i in range(4)]
            vaug_sb = [vp.tile([P, 1024], BF16, tag=f"va{i}", name=f"va{i}") for i in range(11)]
            hoT_sb = [hp_pool.tile([P, N], F16, tag=f"ho{i}", name=f"ho{i}") for i in range(4)]

            # collective bounce buffers
            cc_in = dp.tile([4, P, N], F16, tag="ccin")
            cc_out = dp.tile([4, 2, P, N], F16, tag="ccout")

            # ================= phase 1: QKV + RoPE =================
            with (
                tc.tile_pool(name="ph1in", bufs=1) as ip,
                tc.tile_pool(name="ph1t", bufs=3) as tp,
                tc.tile_pool(name="ps_qk", bufs=3, space="PSUM") as ps_qk,
                tc.tile_pool(name="ps_r", bufs=2, space="PSUM") as ps_r,
                tc.tile_pool(name="ps_v", bufs=3, space="PSUM") as ps_v,
            ):
                xt_sb = [ip.tile([P, N], F16, tag=f"xt{j}", name=f"xt{j}") for j in range(8)]
                wq_sb = [ip.tile([P, 512], F16, tag=f"wq{j}", name=f"wq{j}") for j in range(8)]
                wk_sb = [ip.tile([P, 512], F16, tag=f"wk{j}", name=f"wk{j}") for j in range(8)]
                wv_sb = [ip.tile([P, 512], F16, tag=f"wv{j}", name=f"wv{j}") for j in range(8)]
                for j in range(8):
                    dma(xt_sb[j][:, :], xt_r[j, :, :])
                    dma(wq_sb[j][:, :], wq_r[j, :, :])
                    dma(wk_sb[j][:, :], wk_r[j, :, :])
                    dma(wv_sb[j][:, :], wv_r[j, :, :])

                # v for all 8 heads: [tok, d] tiles + ones columns
                for i, (t0, tw) in enumerate(TOKBLOCKS):
                    nc.gpsimd.memset(vaug_sb[i][:, :], 1.0)
                    v_ps = ps_v.tile([P, 512], F32, tag="v")
                    for j in range(8):
                        nc.tensor.matmul(
                            v_ps[:tw, :],
                            lhsT=xt_sb[j][:, t0 : t0 + tw],
                            rhs=wv_sb[j][:, :],
                            start=(j == 0),
                            stop=False,
                        )
                    nc.tensor.matmul(
                        v_ps[:tw, :],
                        lhsT=ones_sb[0:1, :tw],
                        rhs=bv_sb[:, :],
                        start=False,
                        stop=True,
                    )
                    nc.vector.tensor_copy(
                        vaug_sb[i][:tw].rearrange("p (h c) -> p h c", c=P)[:, :, 0:64],
                        v_ps[:tw].rearrange("p (h c) -> p h c", c=64),
                    )

                # q / k per head-pair, then RoPE
                for hp in range(4):
                    for which, w_sb, b_sb, dst in (
                        ("q", wq_sb, bq_sb, qb_sb),
                        ("k", wk_sb, bk_sb, kb_sb),
                    ):
                        for c0, cw in CH1370:
                            ps = ps_qk.tile([P, 512], F32, tag="qk", name="psqk")
                            for j in range(8):
                                nc.tensor.matmul(
                                    ps[:, 0:cw],
                                    lhsT=w_sb[j][:, hp * P : (hp + 1) * P],
                                    rhs=xt_sb[j][:, c0 : c0 + cw],
                                    start=(j == 0),
                                    stop=(j == 7),
                                )
                            # evacuate + bias (ACT Identity, per-partition bias)
                            nc.scalar.activation(
                                dst[hp][:, c0 : c0 + cw],
                                ps[:, 0:cw],
                                AF.Identity,
                                bias=b_sb[hp][:, :],
                            )
                        # rotate-half via signed-permutation matmul
                        t1 = tp.tile([P, N - 1], F16, tag="t1")
                        t2 = tp.tile([P, N - 1], F16, tag="t2")
                        for c0, cw in CH1369:
                            rps = ps_r.tile([P, 512], F32, tag="rot", name="psrot")
                            nc.tensor.matmul(
                                rps[:, 0:cw],
                                lhsT=rm_sb[:, :],
                                rhs=dst[hp][:, 1 + c0 : 1 + c0 + cw],
                                start=True,
                                stop=True,
                            )
                            nc.vector.tensor_mul(
                                t1[:, c0 : c0 + cw],
                                rps[:, 0:cw],
                                sin_sb[:, c0 : c0 + cw],
                            )
                        nc.vector.tensor_mul(t2[:, :], dst[hp][:, 1:], cos_sb[:, :])
                        nc.vector.tensor_add(dst[hp][:, 1:], t1[:, :], t2[:, :])

            # ================= phase 2: attention =================
            if phases >= 2:
             with (
                tc.tile_pool(name="es", bufs=6) as esp,
                tc.tile_pool(name="rv", bufs=4) as rvp,
                tc.tile_pool(name="ps_st", bufs=2, space="PSUM") as ps_st,
                tc.tile_pool(name="ps_ot", bufs=2, space="PSUM") as ps_ot,
            ):
                for hp in range(4):
                    for half in range(2):
                        qoff = half * NH
                        ots = [ps_ot.tile([P, NH], F32, tag="ot", name="ot") for _ in range(2)]
                        for i, (t0, tw) in enumerate(TOKBLOCKS):
                            for head in range(2):
                                hoff = head * 64
                                hloc = 2 * hp + head
                                ot = ots[head]
                                st = ps_st.tile([P, NH], F32, tag="st", name="st")
                                for c0, cw in CH685:
                                    nc.tensor.matmul(
                                        st[:tw, c0 : c0 + cw],
                                        lhsT=kb_sb[hp][hoff : hoff + 64, t0 : t0 + tw],
                                        rhs=qb_sb[hp][
                                            hoff : hoff + 64, qoff + c0 : qoff + c0 + cw
                                        ],
                                        start=True,
                                        stop=True,
                                    )
                                es = esp.tile([P, NH], BF16, tag="es", name="es")
                                nc.scalar.activation(
                                    es[:tw, :], st[:tw, :], AF.Exp, scale=0.125
                                )
                                for c0, cw in CH685:
                                    nc.tensor.matmul(
                                        ot[:, c0 : c0 + cw],
                                        lhsT=vaug_sb[i][:tw, hloc * P : (hloc + 1) * P],
                                        rhs=es[:tw, c0 : c0 + cw],
                                        start=(i == 0),
                                        stop=(i == 10),
                                        skip_group_check=True,
                                    )
                        for head in range(2):
                            hoff = head * 64
                            ot = ots[head]
                            rinv = rvp.tile([64, NH], F32, tag="rinv", name="rinv")
                            sums = rvp.tile([64, NH], F32, tag="sums", name="sums")
                            nc.scalar.activation(sums[:, :], ot[64:128, :], AF.Copy)
                            nc.vector.reciprocal_approx_fast(out=rinv[:, :], in_=sums[:, :])
                            for c0, cw in CH685:
                                nc.vector.tensor_mul(
                                    hoT_sb[hp][
                                        hoff : hoff + 64, qoff + c0 : qoff + c0 + cw
                                    ],
                                    ot[0:64, c0 : c0 + cw],
                                    rinv[:, c0 : c0 + cw],
                                )
                    if phases >= 3:
                        dma(cc_in[hp, :, :], hoT_sb[hp][:, :])
                        if not os.environ.get('KERNEL_NO_CC'):
                            nc.gpsimd.collective_compute(
                                "AllGather",
                                mybir.AluOpType.bypass,
                                replica_groups=[[0, 1], [2, 3], [4, 5], [6, 7]],
                                ins=[cc_in[hp, :, :]],
                                outs=[cc_out[hp, :, :, :]],
                            )
                        else:
                            dma(cc_out[hp, 0, :, :], cc_in[hp, :, :])
                            dma(cc_out[hp, 1, :, :], cc_in[hp, :, :])

            # ================= phase 3: projection =================
            if phases >= 3:
             with (
                tc.tile_pool(name="ph3", bufs=1) as p3,
                tc.tile_pool(name="ph3o", bufs=2) as p3o,
                tc.tile_pool(name="ps_pj", bufs=4, space="PSUM") as ps_pj,
            ):
                hg_sb = [p3.tile([P, N], F16, tag=f"hg{j}", name=f"hg{j}") for j in range(8)]
                wp_sb = [p3.tile([P, 512], F16, tag=f"wp{j}", name=f"wp{j}") for j in range(8)]
                for j in range(8):
                    dma(hg_sb[j][:, :], cc_out[j % 4, j // 4, :, :])
                    dma(wp_sb[j][:, :], wp_r[j, :, :])
                for t0, tw in TOKBLOCKS:
                    pj = ps_pj.tile([P, 512], F32, tag="pj")
                    for j in range(8):
                        nc.tensor.matmul(
                            pj[:tw, :],
                            lhsT=hg_sb[j][:, t0 : t0 + tw],
                            rhs=wp_sb[j][:, :],
                            start=(j == 0),
                            stop=False,
                        )
                    nc.tensor.matmul(
                        pj[:tw, :],
                        lhsT=ones_sb[0:1, :tw],
                        rhs=bp_sb[:, :],
                        start=False,
                        stop=True,
                    )
                    o_sb = p3o.tile([P, 512], F32, tag="o")
                    nc.scalar.activation(o_sb[:tw, :], pj[:tw, :], AF.Copy)
                    dma(out_d[t0 : t0 + tw, :], o_sb[:tw, :])

    if phases < 3:
        with tile.TileContext(nc) as tc2:
            with tc2.tile_pool(name="dummy", bufs=1) as dq, tc2.tile_pool(name="dps", bufs=1, space="PSUM") as dps:
                z = dq.tile([P, 512], F32, tag="z")
                nc.gpsimd.memset(z[:, :], 0.0)
                for t0, tw in TOKBLOCKS:
                    dma(out_d[t0 : t0 + tw, :], z[:tw, :])
    if not nc.is_finalized():
        nc.finalize()
    return nc


def _get_nc():
    global _cached_nc
    if _cached_nc is None:
        _cached_nc = _build_nc()
    return _cached_nc


_last_result = None


def _rmat_np():
    m = np.zeros((64, 64), np.float32)
    for i in range(32):
        m[i, i + 32] = -1.0
        m[i + 32, i] = 1.0
    r = np.zeros((128, 128), np.float32)
    r[:64, :64] = m
    r[64:, 64:] = m
    return r.T.astype(np.float16)


def kernel(x, sin, cos, w_qkv, b_qkv, w_proj, b_proj):
    global _last_result
    from concourse.bass_utils import run_bass_kernel_spmd

    x = np.asarray(x, np.float32)
    sin = np.asarray(sin, np.float32)
    cos = np.asarray(cos, np.float32)
    w_qkv = np.asarray(w_qkv, np.float32)
    b_qkv = np.asarray(b_qkv, np.float32)
    w_proj = np.asarray(w_proj, np.float32)
    b_proj = np.asarray(b_proj, np.float32)

    sint = np.ascontiguousarray(np.tile(sin.T, (2, 1))).astype(np.float16)
    cost = np.ascontiguousarray(np.tile(cos.T, (2, 1))).astype(np.float16)
    rmat = _rmat_np()

    in_maps = []
    for core in range(8):
        b, g = core // 2, core % 2
        cs = slice(g * 512, (g + 1) * 512)
        in_maps.append(
            {
                "xt": np.ascontiguousarray(x[b].T).astype(np.float16),
                "wq": np.ascontiguousarray(w_qkv[:, cs]).astype(np.float16),
                "wk": np.ascontiguousarray(w_qkv[:, 1024:][:, cs]).astype(np.float16),
                "wv": np.ascontiguousarray(w_qkv[:, 2048:][:, cs]).astype(np.float16),
                "bq": np.ascontiguousarray(b_qkv[cs]).astype(np.float32).reshape(4, P, 1),
                "bk": np.ascontiguousarray(b_qkv[1024:][cs]).astype(np.float32).reshape(4, P, 1),
                "bv": np.ascontiguousarray(b_qkv[2048:][cs]).astype(np.float16).reshape(1, 512),
                "sint": sint,
                "cost": cost,
                "rmat": rmat,
                "wp": np.ascontiguousarray(w_proj[:, cs]).astype(np.float16),
                "bp": np.ascontiguousarray(b_proj[cs]).astype(np.float16).reshape(1, 512),
            }
        )

    nc = _get_nc()
    res = run_bass_kernel_spmd(nc, in_maps, core_ids=list(range(8)))
    _last_result = res
    out = np.empty((B, N, C), np.float32)
    for core in range(8):
        b, g = core // 2, core % 2
        out[b, :, g * 512 : (g + 1) * 512] = res.results[core]["out"]
    return out



# revision 3
# speedup vs baseline: 2.5054x; 2.5054x over previous
"""Multi-head attention (B=4, N=1370, C=1024, H=16) on 8 TRN2 NeuronCores.

Wall-clock on the axon tunnel is transfer-bound (~50MB/s H2D, ~40MB/s D2H,
device kernel ~1ms), so the design minimizes bytes on the wire:

Sharding: tensor-parallel over heads — core c owns heads (2c, 2c+1) for ALL
batches. Each core uploads only unique data (~2.5MB fp16):
  gx [1024, 685]  x^T token-shard c (flat tokens 685c..685c+685)
  ga [32, 1369]   rows 0-7 sin^T rows 8c..8c+8, rows 8-15 cos^T rows,
                  rows 16-31 (cols 0:128) rmat rows 16c..16c+16
  ws [1025, 512]  cols 0:128 wq | 128:256 wk | 256:384 wv | 384:512 wp
                  (col-slices 128c..128c+128 of each), row 1024 = biases
Two on-device AllGathers (all 8 cores) reconstruct full x^T / sin / cos /
rmat on every core over NeuronLink instead of duplicating them on the slow
host link. QKV (fp16 matmuls, fp32 accum, bias via rank-1 ones matmul),
RoPE via signed-permutation matmul + DVE, attention with scores transposed
[ktok, qtok] (softmax-exp ACT PSUM->SBUF, no transposes), denominators via
ones-columns packed into the v stationary operand. Head outputs [128, 5480]
are AllGathered per batch, then each core computes its 128 projection
columns. Output is fp16 [5480, 128] per core (halves D2H + donation-zeros
upload vs fp32).
"""

import numpy as np

B, N, C, H, DH = 4, 1370, 1024, 16, 64
P = 128
NT = B * N        # 5480 flat tokens
SH = NT // 8      # 685 tokens per core shard
NH = 685          # q half per batch

# per-batch k/v token blocks (1370 = 10*128 + 90)
BB = [(i * P, P) for i in range(10)] + [(1280, 90)]
# flat-token chunks for QKV matmul free dim (5480 = 10*512 + 360)
TOKCHUNKS = [(i * 512, 512) for i in range(10)] + [(5120, 360)]
# flat-token blocks for projection partitions (5480 = 42*128 + 104)
PROJBLOCKS = [(i * P, P) for i in range(42)] + [(5376, 104)]
CH685 = [(0, 512), (512, 173)]
CH1369 = [(0, 512), (512, 512), (1024, 345)]

_cached_nc = None


def _build_nc():
    import concourse.bass as bass
    import concourse.mybir as mybir
    import concourse.tile as tile
    from concourse import bacc

    mdt = mybir.dt
    F16, F32, BF16 = mdt.float16, mdt.float32, mdt.bfloat16
    AF = mybir.ActivationFunctionType
    ALL8 = [[0, 1, 2, 3, 4, 5, 6, 7]]

    nc = bacc.Bacc(num_devices=8)

    gx_d = nc.declare_dram_parameter("gx", [C, SH], F16, isOutput=False)
    ga_d = nc.declare_dram_parameter("ga", [32, 1369], F16, isOutput=False)
    ws_d = nc.declare_dram_parameter("ws", [1025, 512], F16, isOutput=False)
    out_d = nc.declare_dram_parameter("out", [NT, P], F16, isOutput=True)

    _dma_engines = [nc.sync, nc.gpsimd, nc.scalar, nc.sync, nc.gpsimd]
    _dma_i = [0]

    def dma(out_ap, in_ap):
        e = _dma_engines[_dma_i[0] % len(_dma_engines)]
        _dma_i[0] += 1
        e.dma_start(out_ap, in_ap)

    with tile.TileContext(nc) as tc:
        with (
            tc.tile_pool(name="dram", bufs=1, space="DRAM") as dp,
            tc.tile_pool(name="const", bufs=1) as cp,
            tc.tile_pool(name="qkv", bufs=1) as qp,
            tc.tile_pool(name="vaug", bufs=1) as vp,
            tc.tile_pool(name="hot", bufs=1) as hp_pool,
        ):
            gatx = dp.tile([8, C, SH], F16, tag="gatx")
            gata = dp.tile([8, 32, 1369], F16, tag="gata")
            cc_in = dp.tile([B, P, N], F16, tag="ccin")
            cc_out = dp.tile([B, 8, P, N], F16, tag="ccout")
            # HW verifier: collectives cannot read IO tensors — bounce params
            # through internal DRAM tiles first (HBM->HBM DMA).
            gxc = dp.tile([C, SH], F16, tag="gxc")
            gac = dp.tile([32, 1369], F16, tag="gac")

            nc.sync.dma_start(gxc[:, :], gx_d[:, :])
            nc.scalar.dma_start(gac[:, :], ga_d[:, :])
            nc.gpsimd.collective_compute(
                "AllGather", mybir.AluOpType.bypass, replica_groups=ALL8,
                ins=[gxc[:, :]], outs=[gatx[:, :, :]],
            )
            nc.gpsimd.collective_compute(
                "AllGather", mybir.AluOpType.bypass, replica_groups=ALL8,
                ins=[gac[:, :]], outs=[gata[:, :, :]],
            )

            # ---- constants ----
            sin_sb = cp.tile([P, 1369], F16, tag="sin")
            cos_sb = cp.tile([P, 1369], F16, tag="cos")
            rm_sb = cp.tile([P, P], F16, tag="rm")
            wall_sb = [cp.tile([P, 512], F16, tag=f"w{j}", name=f"w{j}") for j in range(8)]
            b_sb = cp.tile([1, 512], F16, tag="b")
            ones_sb = cp.tile([1, NT], F16, tag="ones")
            nc.gpsimd.memset(ones_sb[:, :], 1.0)
            for j in range(8):
                dma(wall_sb[j][:, :], ws_d[P * j : P * (j + 1), :])
            dma(b_sb[:, :], ws_d[1024:1025, :])
            for s in range(8):
                dma(sin_sb[8 * s : 8 * s + 8, :], gata[s, 0:8, :])
                dma(sin_sb[64 + 8 * s : 64 + 8 * s + 8, :], gata[s, 0:8, :])
                dma(cos_sb[8 * s : 8 * s + 8, :], gata[s, 8:16, :])
                dma(cos_sb[64 + 8 * s : 64 + 8 * s + 8, :], gata[s, 8:16, :])
                dma(rm_sb[16 * s : 16 * s + 16, :], gata[s, 16:32, 0:P])

            # persistent activations
            qb_sb = qp.tile([P, NT], F16, tag="qb", name="qb")
            kb_sb = qp.tile([P, NT], F16, tag="kb", name="kb")
            vaug_sb = [vp.tile([P, 256], BF16, tag=f"va{i}", name=f"va{i}") for i in range(4 * len(BB))]
            hoT_sb = hp_pool.tile([P, NT], F16, tag="ho", name="ho")

            # ================= phase 1: QKV + RoPE =================
            with (
                tc.tile_pool(name="ph1in", bufs=1) as ip,
                tc.tile_pool(name="ph1t", bufs=3) as tp,
                tc.tile_pool(name="ps_qk", bufs=3, space="PSUM") as ps_qk,
                tc.tile_pool(name="ps_r", bufs=2, space="PSUM") as ps_r,
                tc.tile_pool(name="ps_v", bufs=3, space="PSUM") as ps_v,
            ):
                xt_sb = [ip.tile([P, NT], F16, tag=f"xt{j}", name=f"xt{j}") for j in range(8)]
                for j in range(8):
                    for s in range(8):
                        dma(xt_sb[j][:, SH * s : SH * (s + 1)], gatx[s, P * j : P * (j + 1), :])

                # q / k in [qch, tok] orientation, bias via rank-1 ones matmul
                for col0, dst in ((0, qb_sb), (P, kb_sb)):
                    for tc0, tcw in TOKCHUNKS:
                        ps = ps_qk.tile([P, 512], F32, tag="qk", name="psqk")
                        for j in range(8):
                            nc.tensor.matmul(
                                ps[:, 0:tcw],
                                lhsT=wall_sb[j][:, col0 : col0 + P],
                                rhs=xt_sb[j][:, tc0 : tc0 + tcw],
                                start=(j == 0),
                                stop=False,
                            )
                        nc.tensor.matmul(
                            ps[:, 0:tcw],
                            lhsT=b_sb[0:1, col0 : col0 + P],
                            rhs=ones_sb[0:1, tc0 : tc0 + tcw],
                            start=False,
                            stop=True,
                        )
                        nc.scalar.activation(dst[:, tc0 : tc0 + tcw], ps[:, 0:tcw], AF.Copy)

                # v in [tok, vch] orientation + ones cols for denominators
                for b in range(4):
                    for i, (t0, tw) in enumerate(BB):
                        g0 = N * b + t0
                        vi = len(BB) * b + i
                        nc.gpsimd.memset(vaug_sb[vi][:, :], 1.0)
                        v_ps = ps_v.tile([P, P], F32, tag="v")
                        for j in range(8):
                            nc.tensor.matmul(
                                v_ps[:tw, :],
                                lhsT=xt_sb[j][:, g0 : g0 + tw],
                                rhs=wall_sb[j][:, 256:384],
                                start=(j == 0),
                                stop=False,
                            )
                        nc.tensor.matmul(
                            v_ps[:tw, :],
                            lhsT=ones_sb[0:1, 0:tw],
                            rhs=b_sb[0:1, 256:384],
                            start=False,
                            stop=True,
                        )
                        nc.vector.tensor_copy(
                            vaug_sb[vi][:tw].rearrange("p (h c) -> p h c", c=P)[:, :, 0:DH],
                            v_ps[:tw].rearrange("p (h c) -> p h c", c=DH),
                        )

                # RoPE (tokens 1..1369 of each batch)
                for dst in (qb_sb, kb_sb):
                    for b in range(4):
                        base = N * b + 1
                        t1 = tp.tile([P, 1369], F16, tag="t1")
                        t2 = tp.tile([P, 1369], F16, tag="t2")
                        for c0, cw in CH1369:
                            rps = ps_r.tile([P, 512], F32, tag="rot", name="psrot")
                            nc.tensor.matmul(
                                rps[:, 0:cw],
                                lhsT=rm_sb[:, :],
                                rhs=dst[:, base + c0 : base + c0 + cw],
                                start=True,
                                stop=True,
                            )
                            nc.vector.tensor_mul(
                                t1[:, c0 : c0 + cw], rps[:, 0:cw], sin_sb[:, c0 : c0 + cw]
                            )
                        nc.vector.tensor_mul(t2[:, :], dst[:, base : base + 1369], cos_sb[:, :])
                        nc.vector.tensor_add(dst[:, base : base + 1369], t1[:, :], t2[:, :])

            # ================= phase 2: attention =================
            with (
                tc.tile_pool(name="es", bufs=6) as esp,
                tc.tile_pool(name="rv", bufs=4) as rvp,
                tc.tile_pool(name="ps_st", bufs=2, space="PSUM") as ps_st,
                tc.tile_pool(name="ps_ot", bufs=2, space="PSUM") as ps_ot,
            ):
                for b in range(4):
                    tb = N * b
                    for half in range(2):
                        qoff = tb + NH * half
                        ots = [ps_ot.tile([P, NH], F32, tag="ot", name="ot") for _ in range(2)]
                        for i, (t0, tw) in enumerate(BB):
                            g0 = tb + t0
                            vi = len(BB) * b + i
                            for head in range(2):
                                hoff = head * DH
                                ot = ots[head]
                                st = ps_st.tile([P, NH], F32, tag="st", name="st")
                                for c0, cw in CH685:
                                    nc.tensor.matmul(
                                        st[:tw, c0 : c0 + cw],
                                        lhsT=kb_sb[hoff : hoff + DH, g0 : g0 + tw],
                                        rhs=qb_sb[hoff : hoff + DH, qoff + c0 : qoff + c0 + cw],
                                        start=True,
                                        stop=True,
                                    )
                                es = esp.tile([P, NH], BF16, tag="es", name="es")
                                nc.scalar.activation(es[:tw, :], st[:tw, :], AF.Exp, scale=0.125)
                                for c0, cw in CH685:
                                    nc.tensor.matmul(
                                        ot[:, c0 : c0 + cw],
                                        lhsT=vaug_sb[vi][:tw, head * P : (head + 1) * P],
                                        rhs=es[:tw, c0 : c0 + cw],
                                        start=(i == 0),
                                        stop=(i == len(BB) - 1),
                                        skip_group_check=True,
                                    )
                        for head in range(2):
                            hoff = head * DH
                            ot = ots[head]
                            rinv = rvp.tile([DH, NH], F32, tag="rinv", name="rinv")
                            sums = rvp.tile([DH, NH], F32, tag="sums", name="sums")
                            nc.scalar.activation(sums[:, :], ot[DH:P, :], AF.Copy)
                            nc.vector.reciprocal_approx_fast(out=rinv[:, :], in_=sums[:, :])
                            for c0, cw in CH685:
                                nc.vector.tensor_mul(
                                    hoT_sb[hoff : hoff + DH, qoff + c0 : qoff + c0 + cw],
                                    ot[0:DH, c0 : c0 + cw],
                                    rinv[:, c0 : c0 + cw],
                                )
                    dma(cc_in[b, :, :], hoT_sb[:, tb : tb + N])
                    nc.gpsimd.collective_compute(
                        "AllGather", mybir.AluOpType.bypass, replica_groups=ALL8,
                        ins=[cc_in[b, :, :]], outs=[cc_out[b, :, :, :]],
                    )

            # ================= phase 3: projection =================
            with (
                tc.tile_pool(name="ph3", bufs=1) as p3,
                tc.tile_pool(name="ph3o", bufs=2) as p3o,
                tc.tile_pool(name="ps_pj", bufs=4, space="PSUM") as ps_pj,
            ):
                hg_sb = [p3.tile([P, NT], F16, tag=f"hg{s}", name=f"hg{s}") for s in range(8)]
                for s in range(8):
                    for b in range(4):
                        dma(hg_sb[s][:, N * b : N * (b + 1)], cc_out[b, s, :, :])
                for t0, tw in PROJBLOCKS:
                    pj = ps_pj.tile([P, P], F32, tag="pj")
                    for s in range(8):
                        nc.tensor.matmul(
                            pj[:tw, :],
                            lhsT=hg_sb[s][:, t0 : t0 + tw],
                            rhs=wall_sb[s][:, 384:512],
                            start=(s == 0),
                            stop=False,
                        )
                    nc.tensor.matmul(
                        pj[:tw, :],
                        lhsT=ones_sb[0:1, 0:tw],
                        rhs=b_sb[0:1, 384:512],
                        start=False,
                        stop=True,
                    )
                    o_sb = p3o.tile([P, P], F16, tag="o")
                    nc.scalar.activation(o_sb[:tw, :], pj[:tw, :], AF.Copy)
                    dma(out_d[t0 : t0 + tw, :], o_sb[:tw, :])

    if not nc.is_finalized():
        nc.finalize()
    return nc


def _get_nc():
    global _cached_nc
    if _cached_nc is None:
        _cached_nc = _build_nc()
    return _cached_nc


def _rmat_np():
    m = np.zeros((64, 64), np.float32)
    for i in range(32):
        m[i, i + 32] = -1.0
        m[i + 32, i] = 1.0
    r = np.zeros((128, 128), np.float32)
    r[:64, :64] = m
    r[64:, 64:] = m
    return r.T.astype(np.float16)


_bufs = None


def _get_bufs():
    global _bufs
    if _bufs is None:
        _bufs = {
            "gx": np.zeros((8, C, SH), np.float16),
            "ga": np.zeros((8, 32, 1369), np.float16),
            "ws": np.zeros((8, 1025, 512), np.float16),
            "out": np.empty((B, N, C), np.float32),
        }
    return _bufs


def build_in_maps(x, sin, cos, w_qkv, b_qkv, w_proj, b_proj):
    x = np.asarray(x, np.float32)
    sin = np.asarray(sin, np.float32)
    cos = np.asarray(cos, np.float32)
    w_qkv = np.asarray(w_qkv, np.float32)
    b_qkv = np.asarray(b_qkv, np.float32)
    w_proj = np.asarray(w_proj, np.float32)
    b_proj = np.asarray(b_proj, np.float32)

    bufs = _get_bufs()
    gx, ga, ws = bufs["gx"], bufs["ga"], bufs["ws"]

    xf = x.reshape(NT, C)
    for c in range(8):
        np.copyto(gx[c], xf[SH * c : SH * (c + 1)].T, casting="unsafe")

    # sin/cos tokens are padded (N_ROPE=1369) rows of sin^T/cos^T; rmat chunks
    ga[:, 0:8, :] = sin.T.reshape(8, 8, 1369)
    ga[:, 8:16, :] = cos.T.reshape(8, 8, 1369)
    ga[:, 16:32, 0:P] = _rmat_np().reshape(8, 16, P)

    wq3 = w_qkv.reshape(C, 3, 8, P)
    ws[:, :C, 0:128] = wq3[:, 0].transpose(1, 0, 2)
    ws[:, :C, 128:256] = wq3[:, 1].transpose(1, 0, 2)
    ws[:, :C, 256:384] = wq3[:, 2].transpose(1, 0, 2)
    ws[:, :C, 384:512] = w_proj.reshape(C, 8, P).transpose(1, 0, 2)
    bq3 = b_qkv.reshape(3, 8, P)
    ws[:, C, 0:128] = bq3[0]
    ws[:, C, 128:256] = bq3[1]
    ws[:, C, 256:384] = bq3[2]
    ws[:, C, 384:512] = b_proj.reshape(8, P)

    return [{"gx": gx[c], "ga": ga[c], "ws": ws[c]} for c in range(8)]


def kernel(x, sin, cos, w_qkv, b_qkv, w_proj, b_proj):
    from concourse.bass_utils import run_bass_kernel_spmd

    in_maps = build_in_maps(x, sin, cos, w_qkv, b_qkv, w_proj, b_proj)
    nc = _get_nc()
    res = run_bass_kernel_spmd(nc, in_maps, core_ids=list(range(8)))
    out = _get_bufs()["out"]
    for c in range(8):
        out[:, :, P * c : P * (c + 1)] = res.results[c]["out"].reshape(B, N, P)
    return out
